# revision 21
# baseline (speedup 1.0000x reference)
"""Maxish pooling kernel for Trainium2 (8 NeuronCores, data-parallel).

Reference math (per row of length N, s == 1):
    m  = max(x)
    r  = max(1/(m + 1e-8), 0)          # clamp keeps exponents <= ~1
    v  = exp(r * x)                     # bias-free: v = u * e^{rm}
    out = m * e^{-rm} * sum(v^2) / sum(v)
This single formula covers all three reference branches:
    m > 0: pos = m * sum(u^2)/sum(u)   (u = exp(r(x-m)))
    m < 0: r = 0 -> v = 1 -> out = m   (the neg branch value)
    m = 0: r huge, rm = 0, S1/S2 finite -> out = 0

Layout: rows on partitions, SWDGE DMA casts f32->bf16 with 16 KiB
contiguous per-partition runs. Row max via bf16 TT-max tree (2x mode)
+ 1x tensor_reduce tail. exp per tile on ACT (scale=r [P,1]). Sums per
tile on DVE: bn_stats, or ts+accum (sum v) + ttr (sum v^2). The chunk
loop is software-pipelined: chunk c+1's max tree and smalls are issued
to DVE before chunk c's sums, so DVE never stalls on ACT's exps.
"""

import numpy as np

P = 128
N = 256
SMALL = 1e-8


def _build(n_rows: int, s: float, G: int = 16, w2: int = 0, levels: int = 3,
           sums: str = "bn2", fgrp: int = 8, x_bufs: int = 3,
           v_bufs: int = 3, cast: bool = True, fastrecip: bool = True,
           gsl1: int = 0, w2gs: bool = False):
    from concourse import bacc, mybir
    from concourse.tile import TileContext

    f32 = mybir.dt.float32
    bf16 = mybir.dt.bfloat16
    Act = mybir.ActivationFunctionType
    Alu = mybir.AluOpType
    Ax = mybir.AxisListType

    assert n_rows % (P * G) == 0
    T = n_rows // P          # tiles of [128, N]
    C = T // G               # chunks of G tiles
    assert s == 1.0, "fast path only"
    w2 = min(w2, G)          # tiles per chunk on the ts-mult + big-exp route
    if sums == "bn2":
        assert G % 2 == 0 and w2 % 2 == 0
    dt_x = bf16 if cast else f32

    nc = bacc.Bacc("TRN2", target_bir_lowering=False, debug=False,
                   num_devices=8)
    x_d = nc.declare_dram_parameter("x", [n_rows, N], f32, isOutput=False)
    out_d = nc.declare_dram_parameter("out", [n_rows], f32, isOutput=True)

    with TileContext(nc) as tc:
        with (
            tc.tile_pool(name="xp", bufs=x_bufs) as xp,
            tc.tile_pool(name="vp", bufs=v_bufs) as vp,
            tc.tile_pool(name="mxp", bufs=2) as mxp,
            tc.tile_pool(name="bst", bufs=2) as bstp,
            tc.tile_pool(name="wp", bufs=2) as wp,
            tc.tile_pool(name="stat", bufs=1) as statp,
            tc.tile_pool(name="consts", bufs=4) as cpool,
        ):
            M = statp.tile([P, T], f32, tag="M")        # per-row max
            RM = statp.tile([P, T], f32, tag="RM")      # r*m per row
            RG = statp.tile([P, T], f32, tag="RG")      # r per row
            S1 = statp.tile([P, T], f32, tag="S1")      # sum v^2
            S2 = statp.tile([P, T], f32, tag="S2")      # sum v
            R = statp.tile([P, T], f32, tag="R")        # final result
            E = statp.tile([P, T], f32, tag="E")        # exp(-rm)
            T1 = statp.tile([P, T], f32, tag="T1")      # mu_e+mu_o
            T2 = statp.tile([P, T], f32, tag="T2")      # mu_e^2+mu_o^2
            T4 = statp.tile([P, T], f32, tag="T4")      # M2_e+M2_o

            xts, vts = {}, {}

            def load(c):
                xt = xp.tile([P, G * N], dt_x, tag="x")
                xts[c] = xt
                src = x_d[c * G * P:(c + 1) * G * P, :].rearrange(
                    "(p k) n -> p (k n)", p=P)
                if cast:
                    nc.gpsimd.dma_start(out=xt[:], in_=src)
                else:
                    nc.sync.dma_start(out=xt[:], in_=src)

            def head(c):
                # max tree + smalls for chunk c (DVE + GS work, no ACT dep)
                xt = xts[c]
                x3 = xt[:].rearrange("p (g n) -> p g n", n=N)
                mg = M[:, c * G:(c + 1) * G]
                if cast and levels > 0:
                    h = N // 2
                    mx = mxp.tile([P, G * h], dt_x, tag="mx")
                    m3 = mx[:].rearrange("p (g n) -> p g n", n=h)
                    if gsl1 > 0:
                        nc.vector.tensor_tensor(
                            m3[:, gsl1:], x3[:, gsl1:, 0:h],
                            x3[:, gsl1:, h:N], op=Alu.max)
                        nc.gpsimd.tensor_tensor(
                            m3[:, :gsl1], x3[:, :gsl1, 0:h],
                            x3[:, :gsl1, h:N], op=Alu.max)
                    else:
                        nc.vector.tensor_tensor(
                            m3, x3[:, :, 0:h], x3[:, :, h:N], op=Alu.max)
                    for _ in range(1, levels):
                        h2 = h // 2
                        nc.vector.tensor_tensor(
                            m3[:, :, 0:h2], m3[:, :, 0:h2], m3[:, :, h2:h],
                            op=Alu.max)
                        h = h2
                    nc.vector.tensor_reduce(out=mg, in_=m3[:, :, 0:h],
                                            axis=Ax.X, op=Alu.max)
                else:
                    nc.vector.tensor_reduce(out=mg, in_=x3, axis=Ax.X,
                                            op=Alu.max)
                # r = clamp(1/m, >=0): kept DVE-local so ACT's exps aren't
                # gated on a GS round-trip. 1/m vs 1/(m+eps) differs by
                # ~4e-9 rel for the |m|>1 rows that matter; m<=0 rows give
                # NaN/negative -> clamped to 0 (DVE max(NaN,0)=0).
                rg = RG[:, c * G:(c + 1) * G]
                if fastrecip:
                    nc.vector.reciprocal_approx_fast(rg, mg)
                else:
                    nc.vector.tensor_scalar_add(rg, mg, SMALL)
                    nc.vector.reciprocal(rg, rg)
                nc.vector.tensor_scalar_max(rg, rg, 0.0)
                rmg = RM[:, c * G:(c + 1) * G]
                nc.gpsimd.tensor_tensor(rmg, mg, rg, op=Alu.mult)

            def exps(c):
                # ACT: v = exp(r*x) per tile; last w2 tiles go ts-mult+big-exp
                xt = xts[c]
                vt = vp.tile([P, G * N], dt_x, tag="v")
                vts[c] = vt
                rg = RG[:, c * G:(c + 1) * G]
                for g in range(G - w2):
                    fs = slice(g * N, (g + 1) * N)
                    nc.scalar.activation(
                        out=vt[:, fs], in_=xt[:, fs], func=Act.Exp,
                        scale=rg[:, g:g + 1])
                if w2:
                    wt = wp.tile([P, w2 * N], dt_x, tag="w")
                    for i, g in enumerate(range(G - w2, G)):
                        fs = slice(g * N, (g + 1) * N)
                        eng = nc.gpsimd if w2gs else nc.vector
                        eng.tensor_scalar(
                            wt[:, i * N:(i + 1) * N], xt[:, fs],
                            rg[:, g:g + 1], None, op0=Alu.mult)
                    bs = slice((G - w2) * N, G * N)
                    nc.scalar.activation(out=vt[:, bs], in_=wt[:],
                                         func=Act.Exp)

            def tails(c):
                # DVE sums for chunk c (depends on ACT's vt)
                vt = vts.pop(c)
                xts.pop(c)
                v3 = vt[:].rearrange("p (g n) -> p g n", n=N)
                s1c = S1[:, c * G:(c + 1) * G]
                s2c = S2[:, c * G:(c + 1) * G]
                if sums == "bn2":
                    # one bn_stats per tile pair, read through an AP that
                    # interleaves the two contiguous tiles element-wise
                    # (A0,B0,A1,B1...): the even stats bank then holds tile
                    # 2j, the odd bank tile 2j+1. Strided reads are free at
                    # bn's 1x rate; ACT keeps fast contiguous writes.
                    bt = bstp.tile([P, (G // 2) * 6], f32, tag="bst")
                    b3 = bt[:].rearrange("p (j s) -> p j s", s=6)
                    vi = vt[:].rearrange("p (j two n) -> p j n two",
                                         two=2, n=N)
                    for j in range(G // 2):
                        # emit InstBNStats directly: in AP [P,256,2] streams
                        # A0,B0,A1,B1... (innermost dim = pair lane); the
                        # bass wrapper would reject the 3D shape but the
                        # engine reduces the whole per-partition stream.
                        nc.vector.add_instruction(mybir.InstBNStats(
                            name=nc.vector.bass.get_next_instruction_name(),
                            ins=[nc.vector.lower_ap(vi[:, j])],
                            outs=[nc.vector.lower_ap(b3[:, j, :])]))
                    # T1[tile] = mu, T4[tile] = M2 (per-row, n=256)
                    bsg = bt[:].rearrange("p (j s) -> p s j", s=6)
                    t1v = T1[:, c * G:(c + 1) * G].rearrange(
                        "p (j two) -> p two j", two=2)
                    t4v = T4[:, c * G:(c + 1) * G].rearrange(
                        "p (j two) -> p two j", two=2)
                    nc.gpsimd.tensor_scalar_mul(t1v[:, 0], bsg[:, 1], 1.0)
                    nc.gpsimd.tensor_scalar_mul(t1v[:, 1], bsg[:, 4], 1.0)
                    nc.gpsimd.tensor_scalar_mul(t4v[:, 0], bsg[:, 2], 1.0)
                    nc.gpsimd.tensor_scalar_mul(t4v[:, 1], bsg[:, 5], 1.0)
                elif sums == "bn":
                    bt = bstp.tile([P, G * 6], f32, tag="bst")
                    b3 = bt[:].rearrange("p (g s) -> p g s", s=6)
                    for g in range(G):
                        nc.vector.bn_stats(out=b3[:, g, :], in_=v3[:, g, :])
                    # sum v = 128(mu_e+mu_o); sum v^2 = M2s + 128(mu^2s);
                    # only accumulate T1/T2/T4 here, combined in final()
                    bsg = bt[:].rearrange("p (g s) -> p s g", s=6)
                    mu_e, m2_e = bsg[:, 1], bsg[:, 2]
                    mu_o, m2_o = bsg[:, 4], bsg[:, 5]
                    cs = slice(c * G, (c + 1) * G)
                    cb = cpool.tile([P, 2 * G], f32, tag="cb")
                    t2 = cb[:, 0:G]
                    t3 = cb[:, G:2 * G]
                    nc.gpsimd.tensor_tensor(T1[:, cs], mu_e, mu_o, op=Alu.add)
                    nc.gpsimd.tensor_tensor(t2, mu_e, mu_e, op=Alu.mult)
                    nc.gpsimd.tensor_tensor(t3, mu_o, mu_o, op=Alu.mult)
                    nc.gpsimd.tensor_tensor(T2[:, cs], t2, t3, op=Alu.add)
                    nc.gpsimd.tensor_tensor(T4[:, cs], m2_e, m2_o, op=Alu.add)
                else:
                    for g in range(G):
                        fs = slice(g * N, (g + 1) * N)
                        nc.vector.tensor_scalar(
                            vt[:, fs], vt[:, fs], 1.0, None, op0=Alu.mult,
                            op1=Alu.add, accum_out=s2c[:, g:g + 1])
                        nc.vector.tensor_tensor_reduce(
                            out=vt[:, fs], in0=vt[:, fs], in1=vt[:, fs],
                            scale=1.0, scalar=0.0, op0=Alu.mult, op1=Alu.add,
                            accum_out=s1c[:, g:g + 1])

            def final(c0, c1):
                # out = M * exp(-RM) * S1/S2 for chunks [c0, c1)
                cs = slice(c0 * G, c1 * G)
                nc.scalar.activation(out=E[:, cs], in_=RM[:, cs],
                                     func=Act.Exp, scale=-1.0)
                if sums == "bn2":
                    # S1/S2 = (M2 + 256 mu^2)/(256 mu) = (mu^2 + M2/256)/mu
                    cb = cpool.tile([P, fgrp * G], f32, tag="fcb")
                    nc.vector.tensor_tensor(cb[:], T1[:, cs], T1[:, cs],
                                            op=Alu.mult)
                    nc.vector.scalar_tensor_tensor(
                        out=S1[:, cs], in0=T4[:, cs], scalar=1.0 / N,
                        in1=cb[:], op0=Alu.mult, op1=Alu.add)
                    nc.vector.reciprocal_approx_fast(S2[:, cs], T1[:, cs])
                elif sums == "bn":
                    # S1/S2 = (T2 + T4/128) / T1
                    nc.vector.scalar_tensor_tensor(
                        out=S1[:, cs], in0=T4[:, cs], scalar=1.0 / (N // 2),
                        in1=T2[:, cs], op0=Alu.mult, op1=Alu.add)
                    nc.vector.reciprocal_approx_fast(S2[:, cs], T1[:, cs])
                else:
                    nc.vector.reciprocal_approx_fast(S2[:, cs], S2[:, cs])
                nc.vector.tensor_tensor(S1[:, cs], S1[:, cs], S2[:, cs],
                                        op=Alu.mult)
                nc.vector.tensor_tensor(S1[:, cs], S1[:, cs], E[:, cs],
                                        op=Alu.mult)
                nc.vector.tensor_tensor(R[:, cs], S1[:, cs], M[:, cs],
                                        op=Alu.mult)
                nc.sync.dma_start(
                    out=out_d[c0 * G * P:c1 * G * P].rearrange(
                        "(c p g) -> p c g", p=P, g=G),
                    in_=R[:, cs].rearrange("p (c g) -> p c g", g=G))

            # software-pipelined schedule
            load(0)
            load(1)
            head(0)
            exps(0)
            for c in range(C):
                if c + 2 < C:
                    load(c + 2)
                if c + 1 < C:
                    head(c + 1)
                    exps(c + 1)
                tails(c)
                if (c + 1) % fgrp == 0:
                    final(c + 1 - fgrp, c + 1)
            if C % fgrp:
                final(C - C % fgrp, C)

    nc.compile()
    return nc


def _run(x: np.ndarray, scale: np.ndarray, trace: bool = False,
         build_kw: dict | None = None, **kw):
    from concourse.bass_utils import run_bass_kernel_spmd

    n_cores = 8
    B, Tm, X, Nn = x.shape          # 32, 256, 64, 256
    assert Nn == N
    rows = B * Tm * X
    rows_per_core = rows // n_cores
    s = float(np.asarray(scale))

    nc = _build(rows_per_core, s, **(build_kw or {}))
    xs = np.ascontiguousarray(np.asarray(x, dtype=np.float32)).reshape(
        n_cores, rows_per_core, N)
    in_maps = [{"x": xs[i]} for i in range(n_cores)]
    res = run_bass_kernel_spmd(nc, in_maps, list(range(n_cores)),
                               trace=trace, **kw)
    out = np.concatenate([r["out"].reshape(-1) for r in res.results], axis=0)
    return out.reshape(B, Tm, X).astype(np.float32), res


def kernel(x: np.ndarray, scale: np.ndarray) -> np.ndarray:
    return _run(x, scale)[0]
